# revision 19
# baseline (speedup 1.0000x reference)
"""CollisionLoss kernel for Trainium2 (8 NeuronCores, Bass/Tile).

Computes: sum over (future, box) of masked AABB-overlap area between the
ego box (per-future, derived from sdc trajectory) and 1M gt boxes per
future, times WEIGHT.

Strategy (memory-bound problem):
 - Host computes the 6 per-future ego AABBs (24 scalars) exactly as the
   reference does (O(1) work).
 - The big tensors (future_gt_corners [6,1M,4,2] f32 = 192 MB, box_mask
   [6,1M] = 6 MB) are sharded along the boxes axis across 8 cores
   (125000 boxes/future/core) with zero-copy numpy views.
 - Each core streams its 24.75 MB through SBUF once and computes a
   per-partition partial sum; host sums the 8x125 partials.

Per-core data layout: each future's [125000, 8]-float corner block is
viewed as [125 partitions, 1000 boxes] (8 floats per box contiguous),
processed in 4 column subtiles of 250 boxes.

Engine split per subtile (boxes b, corners k, coords x/y):
 - DVE:  L1 min/max tree (4 tensor_tensor ops, strided corner views ->
         contiguous planes), negx/negy scalar_tensor_tensor, final
         tensor_tensor_reduce (area product + accumulate).
 - GpSimd: L2 min/max (4 contiguous tensor_tensor), mask-biased
         px (scalar_tensor_tensor), py (tensor_scalar_min).
 - ACT:  mask u8 -> {0, -1e30} cast, relu(-negx), relu(-negy).

Identity used:  w = relu(min(xa1,xb1) - max(xa2,xb2))
  pxm  = min(xb1, xa1) + maskm          (maskm = 0 valid / -1e30 masked)
  negx = max(xb2, xa2) - pxm            ( = -w_raw - maskm )
  wpos = relu(-negx)                    ( = relu(w_raw), 0 when masked )
  area = wpos * hpos ; accumulated per partition by tensor_tensor_reduce.
"""

import numpy as np

DELTA = 0.5
WEIGHT = 1.0
W = 1.85 + DELTA
H = 4.084 + DELTA

F = 6
N = 1_000_000
CORES = 8
PER_CORE = N // CORES  # 125000
P = 125                # SBUF partitions used
BPR = PER_CORE // P    # boxes per partition row = 1000
SUB = 2                # column subtiles per future
B = BPR // SUB         # boxes per subtile column block

_prog = None
_last_in_maps = None


def _build_program(n_fut=F, p=P, bpr=BPR, sub=SUB, cbufs=3, l1bufs=3, sbufs=3, bf16=True):
    from contextlib import ExitStack

    import concourse.bacc as bacc
    import concourse.tile as tile
    from concourse import mybir

    Alu = mybir.AluOpType
    Act = mybir.ActivationFunctionType
    f32 = mybir.dt.float32
    u8 = mybir.dt.uint8
    mid = mybir.dt.bfloat16 if bf16 else f32

    b = bpr // sub
    nc = bacc.Bacc("TRN2", target_bir_lowering=False, debug=False)

    corners = [
        nc.dram_tensor(f"corners{f}", [p * bpr, 8], f32, kind="ExternalInput")
        for f in range(n_fut)
    ]
    masks = [
        nc.dram_tensor(f"mask{f}", [p * bpr], u8, kind="ExternalInput")
        for f in range(n_fut)
    ]
    ego = nc.dram_tensor("ego", [p, 4 * n_fut], f32, kind="ExternalInput")
    out = nc.dram_tensor("out", [p, 1], f32, kind="ExternalOutput")

    with tile.TileContext(nc) as tc, ExitStack() as ctx:
        const_pool = ctx.enter_context(tc.tile_pool(name="const", bufs=1))
        cpool = ctx.enter_context(tc.tile_pool(name="cd", bufs=cbufs))
        mpool = ctx.enter_context(tc.tile_pool(name="mask", bufs=2))
        l1pool = ctx.enter_context(tc.tile_pool(name="l1", bufs=l1bufs))
        spool = ctx.enter_context(tc.tile_pool(name="small", bufs=sbufs))

        ego_sb = const_pool.tile([p, 4 * n_fut], f32)
        nc.sync.dma_start(out=ego_sb[:], in_=ego.ap())
        acc = const_pool.tile([p, n_fut * sub], f32)

        n_tiles = n_fut * sub
        state = {}

        def ego_col(f, k):
            return ego_sb[:, 4 * f + k : 4 * f + k + 1]

        # DMA issue: the issuing sequencer is held for the whole transfer,
        # so one engine alone caps DMA throughput at transfer+setup per
        # period. SP takes most corner loads; ACT (which has compute slack)
        # takes every 6th plus the small mask loads, so transfers pack
        # back-to-back on the DMA engines.
        def s0_dma(t):
            f, s = divmod(t, sub)
            st = state[t] = {}
            cview = corners[f].ap().rearrange("(p b) c -> p (b c)", p=p)
            cd = cpool.tile([p, b * 8], f32, tag="cd")
            eng = nc.scalar if t % 6 == 0 else nc.sync
            eng.dma_start(out=cd[:], in_=cview[:, s * b * 8 : (s + 1) * b * 8])
            st["cd"] = cd
            if s == 0:
                mview = masks[f].ap().rearrange("(p b) -> p b", p=p)
                mtile = mpool.tile([p, bpr], u8, tag="mask")
                nc.scalar.dma_start(out=mtile[:], in_=mview)
                state[("m", f)] = mtile

        def s1_l1(t):
            st = state[t]
            cdh = st["cd"][:].rearrange("p (b h four) -> p b h four", h=2, four=4)
            # L1: one max + one min over the two 4-float half-boxes.
            # Outputs land as 4 contiguous [p, b] planes:
            # k=0 m(x0,x2), k=1 m(y0,y2), k=2 m(x1,x3), k=3 m(y1,y3).
            mx = l1pool.tile([p, 4 * b], mid, tag="mx")
            mn = l1pool.tile([p, 4 * b], mid, tag="mn")
            lo = cdh[:, :, 0, :]
            hi = cdh[:, :, 1, :]
            nc.vector.tensor_tensor(
                out=mx[:].rearrange("p (k b) -> p b k", k=4), in0=lo, in1=hi,
                op=Alu.max,
            )
            nc.vector.tensor_tensor(
                out=mn[:].rearrange("p (k b) -> p b k", k=4), in0=lo, in1=hi,
                op=Alu.min,
            )
            st["mx"], st["mn"] = mx, mn

        def s2_l2(t):
            f, s = divmod(t, sub)
            st = state[t]
            mx, mn = st["mx"], st["mn"]
            xb1 = spool.tile([p, b], mid, tag="xb1")
            xb2 = spool.tile([p, b], mid, tag="xb2")
            yb1 = spool.tile([p, b], mid, tag="yb1")
            yb2 = spool.tile([p, b], mid, tag="yb2")
            # L2 must be on DVE: walrus rejects 2-tensor ops on Pool.
            nc.vector.tensor_tensor(
                out=xb1[:], in0=mx[:, 0:b], in1=mx[:, 2 * b : 3 * b], op=Alu.max
            )
            nc.vector.tensor_tensor(
                out=xb2[:], in0=mn[:, 0:b], in1=mn[:, 2 * b : 3 * b], op=Alu.min
            )
            nc.vector.tensor_tensor(
                out=yb1[:], in0=mx[:, b : 2 * b], in1=mx[:, 3 * b : 4 * b], op=Alu.max
            )
            nc.vector.tensor_tensor(
                out=yb2[:], in0=mn[:, b : 2 * b], in1=mn[:, 3 * b : 4 * b], op=Alu.min
            )
            # mask -> {0 valid, -1e30 masked} (ACT, u8 -> f32 cast)
            maskm = spool.tile([p, b], mid, tag="maskm")
            nc.scalar.activation(
                out=maskm[:], in_=state[("m", f)][:, s * b : (s + 1) * b],
                func=Act.Copy, bias=-1e30, scale=1e30,
            )
            st.update(xb1=xb1, xb2=xb2, yb1=yb1, yb2=yb2, maskm=maskm)

        def s3(t):
            f, s = divmod(t, sub)
            st = state[t]
            pxm = spool.tile([p, b], mid, tag="pxm")
            nc.vector.scalar_tensor_tensor(
                out=pxm[:], in0=st["xb1"][:], scalar=ego_col(f, 0),
                in1=st["maskm"][:], op0=Alu.min, op1=Alu.add,
            )
            # py = min(yb1, ya1) — 1-input tensor_scalar is legal on Pool
            py = spool.tile([p, b], mid, tag="py")
            nc.gpsimd.tensor_scalar_min(out=py[:], in0=st["yb1"][:],
                                        scalar1=ego_col(f, 2))
            st.update(pxm=pxm, py=py)

        def s4(t):
            f, s = divmod(t, sub)
            st = state[t]
            negx = spool.tile([p, b], mid, tag="negx")
            nc.vector.scalar_tensor_tensor(
                out=negx[:], in0=st["xb2"][:], scalar=ego_col(f, 1),
                in1=st["pxm"][:], op0=Alu.max, op1=Alu.subtract,
            )
            negy = spool.tile([p, b], mid, tag="negy")
            nc.vector.scalar_tensor_tensor(
                out=negy[:], in0=st["yb2"][:], scalar=ego_col(f, 3),
                in1=st["py"][:], op0=Alu.max, op1=Alu.subtract,
            )
            st.update(negx=negx, negy=negy)

        def s5(t):
            st = state[t]
            hpos = spool.tile([p, b], mid, tag="hpos")
            nc.scalar.activation(out=hpos[:], in_=st["negy"][:], func=Act.Relu,
                                 scale=-1.0)
            st["hpos"] = hpos

        def s6(t):
            st = state[t]
            # area accumulate: (negx min 0) * hpos = -wpos*hpos, summed
            # per partition into acc column (host negates the total).
            scr = spool.tile([p, b], f32, tag="scr")
            nc.vector.scalar_tensor_tensor(
                out=scr[:], in0=st["negx"][:], scalar=0.0, in1=st["hpos"][:],
                op0=Alu.min, op1=Alu.mult,
                accum_out=acc[:, t : t + 1],
            )
            del state[t]

        # 7-stage software pipeline: every cross-engine hop of the tail
        # chain lands in its own period, so no in-order engine queue ever
        # blocks on a same-subtile dependency.
        stages = [s0_dma, s1_l1, s2_l2, s3, s4, s5, s6]
        for t in range(n_tiles + len(stages) - 1):
            for k, fn in enumerate(stages):
                tt = t - k
                if 0 <= tt < n_tiles:
                    fn(tt)

        total = const_pool.tile([p, 1], f32)
        nc.vector.reduce_sum(out=total[:], in_=acc[:], axis=mybir.AxisListType.X)
        nc.sync.dma_start(out=out.ap(), in_=total[:])

    nc.compile()
    return nc


def _get_prog():
    global _prog
    if _prog is None:
        _prog = _build_program()
    return _prog


def _ego_aabb(sdc_traj_all, sdc_planning_gt):
    """Per-future ego AABB [F,4] = (xa1, xa2, ya1, ya2), mirroring reference."""
    sdc_traj_all = np.asarray(sdc_traj_all, dtype=np.float32)
    sdc_planning_gt = np.asarray(sdc_planning_gt, dtype=np.float32)
    x = sdc_traj_all[0, :, 0]
    y = sdc_traj_all[0, :, 1]
    theta = sdc_planning_gt[0, :, 2]
    local = np.array(
        [[W / 2, -H / 2], [W / 2, H / 2], [-W / 2, H / 2], [-W / 2, -H / 2]],
        dtype=np.float32,
    )
    c, s = np.cos(theta), np.sin(theta)
    rot = np.stack([np.stack([c, s], -1), np.stack([-s, c], -1)], -2)  # [F,2,2]
    corners = np.einsum("fij,kj->fki", rot, local) + np.stack([x, y], -1)[:, None, :]
    corners = corners.astype(np.float32)
    xa1 = corners[..., 0].max(-1)
    ya1 = corners[..., 1].max(-1)
    xa2 = corners[..., 0].min(-1)
    ya2 = corners[..., 1].min(-1)
    return np.stack([xa1, xa2, ya1, ya2], -1).astype(np.float32)  # [F,4]


def kernel(sdc_traj_all, sdc_planning_gt, sdc_planning_gt_mask, future_gt_corners, box_mask):
    from concourse.bass_utils import run_bass_kernel_spmd

    corners = np.asarray(future_gt_corners, dtype=np.float32).reshape(F, N, 8)
    mask = np.asarray(box_mask)
    if mask.dtype == np.bool_:
        mask_u8 = mask.view(np.uint8)
    else:
        mask_u8 = (mask != 0).astype(np.uint8)

    ego = _ego_aabb(sdc_traj_all, sdc_planning_gt).reshape(4 * F)
    ego_arr = np.ascontiguousarray(
        np.broadcast_to(ego, (P, 4 * F)), dtype=np.float32
    )

    in_maps = []
    for cidx in range(CORES):
        lo, hi = cidx * PER_CORE, (cidx + 1) * PER_CORE
        m = {"ego": ego_arr}
        for f in range(F):
            m[f"corners{f}"] = corners[f, lo:hi]
            m[f"mask{f}"] = mask_u8[f, lo:hi]
        in_maps.append(m)

    global _last_in_maps
    _last_in_maps = in_maps
    res = run_bass_kernel_spmd(_get_prog(), in_maps, list(range(CORES))).results
    total = 0.0
    for r in res:
        total += float(r["out"].astype(np.float64).sum())
    # device accumulates -area (see area STT); negate here
    return np.array([-total], dtype=np.float32) * np.float32(WEIGHT)


# revision 23
# speedup vs baseline: 1.6727x; 1.6727x over previous
"""CollisionLoss kernel for Trainium2 (8 NeuronCores, Bass/Tile).

Computes: sum over (future, box) of masked AABB-overlap area between the
ego box (per-future, derived from sdc trajectory) and 1M gt boxes per
future, times WEIGHT.

Strategy (memory-bound problem):
 - Host computes the 6 per-future ego AABBs (24 scalars) exactly as the
   reference does (O(1) work).
 - The big tensors (future_gt_corners [6,1M,4,2] f32 = 192 MB, box_mask
   [6,1M] = 6 MB) are sharded along the boxes axis across 8 cores
   (125000 boxes/future/core) with zero-copy numpy views.
 - Each core streams its 24.75 MB through SBUF once and computes a
   per-partition partial sum; host sums the 8x125 partials.

Per-core data layout: each future's [125000, 8]-float corner block is
viewed as [125 partitions, 1000 boxes] (8 floats per box contiguous),
processed in 4 column subtiles of 250 boxes.

Engine split per subtile (boxes b, corners k, coords x/y):
 - DVE:  L1 min/max tree (4 tensor_tensor ops, strided corner views ->
         contiguous planes), negx/negy scalar_tensor_tensor, final
         tensor_tensor_reduce (area product + accumulate).
 - GpSimd: L2 min/max (4 contiguous tensor_tensor), mask-biased
         px (scalar_tensor_tensor), py (tensor_scalar_min).
 - ACT:  mask u8 -> {0, -1e30} cast, relu(-negx), relu(-negy).

Identity used:  w = relu(min(xa1,xb1) - max(xa2,xb2))
  pxm  = min(xb1, xa1) + maskm          (maskm = 0 valid / -1e30 masked)
  negx = max(xb2, xa2) - pxm            ( = -w_raw - maskm )
  wpos = relu(-negx)                    ( = relu(w_raw), 0 when masked )
  area = wpos * hpos ; accumulated per partition by tensor_tensor_reduce.
"""

import numpy as np

DELTA = 0.5
WEIGHT = 1.0
W = 1.85 + DELTA
H = 4.084 + DELTA

F = 6
N = 1_000_000
CORES = 8
PER_CORE = N // CORES  # 125000
P = 125                # SBUF partitions used
BPR = PER_CORE // P    # boxes per partition row = 1000
SUB = 2                # column subtiles per future
B = BPR // SUB         # boxes per subtile column block

_prog = None
_last_in_maps = None


def _build_program(n_fut=F, p=P, bpr=BPR, sub=SUB, cbufs=3, l1bufs=3, sbufs=3, bf16=True, l1_dense=False):
    from contextlib import ExitStack

    import concourse.bacc as bacc
    import concourse.tile as tile
    from concourse import mybir

    Alu = mybir.AluOpType
    Act = mybir.ActivationFunctionType
    f32 = mybir.dt.float32
    u8 = mybir.dt.uint8
    mid = mybir.dt.bfloat16 if bf16 else f32

    b = bpr // sub
    nc = bacc.Bacc("TRN2", target_bir_lowering=False, debug=False)

    corners = [
        nc.dram_tensor(f"corners{f}", [p * bpr, 8], f32, kind="ExternalInput")
        for f in range(n_fut)
    ]
    masks = [
        nc.dram_tensor(f"mask{f}", [p * bpr], u8, kind="ExternalInput")
        for f in range(n_fut)
    ]
    ego = nc.dram_tensor("ego", [p, 4 * n_fut], f32, kind="ExternalInput")
    egob = nc.dram_tensor("egob", [p, 4 * n_fut], mybir.dt.bfloat16 if bf16 else f32, kind="ExternalInput")
    out = nc.dram_tensor("out", [p, 1], f32, kind="ExternalOutput")

    with tile.TileContext(nc) as tc, ExitStack() as ctx:
        const_pool = ctx.enter_context(tc.tile_pool(name="const", bufs=1))
        cpool = ctx.enter_context(tc.tile_pool(name="cd", bufs=cbufs))
        mpool = ctx.enter_context(tc.tile_pool(name="mask", bufs=2))
        l1pool = ctx.enter_context(tc.tile_pool(name="l1", bufs=l1bufs))
        spool = ctx.enter_context(tc.tile_pool(name="small", bufs=sbufs))

        ego_sb = const_pool.tile([p, 4 * n_fut], f32)
        nc.sync.dma_start(out=ego_sb[:], in_=ego.ap())
        egob_sb = const_pool.tile([p, 4 * n_fut], mid)
        nc.sync.dma_start(out=egob_sb[:], in_=egob.ap())
        acc = const_pool.tile([p, n_fut * sub], f32)

        n_tiles = n_fut * sub
        state = {}

        def ego_col(f, k):
            return ego_sb[:, 4 * f + k : 4 * f + k + 1]

        # DMA issue: the issuing sequencer is held for the whole transfer,
        # so one engine alone caps DMA throughput at transfer+setup per
        # period. SP takes most corner loads; ACT (which has compute slack)
        # takes every 6th plus the small mask loads, so transfers pack
        # back-to-back on the DMA engines.
        def s0_dma(t):
            f, s = divmod(t, sub)
            st = state[t] = {}
            cview = corners[f].ap().rearrange("(p b) c -> p (b c)", p=p)
            cd = cpool.tile([p, b * 8], f32, tag="cd")
            eng = nc.scalar if t % 6 == 0 else nc.sync
            eng.dma_start(out=cd[:], in_=cview[:, s * b * 8 : (s + 1) * b * 8])
            st["cd"] = cd
            if s == 0:
                mview = masks[f].ap().rearrange("(p b) -> p b", p=p)
                mtile = mpool.tile([p, bpr], u8, tag="mask")
                nc.scalar.dma_start(out=mtile[:], in_=mview)
                state[("m", f)] = mtile

        def s1_l1(t):
            st = state[t]
            cdh = st["cd"][:].rearrange("p (b h four) -> p b h four", h=2, four=4)
            # L1: one max + one min over the two 4-float half-boxes.
            # Output BOX-MAJOR [p, b, 4] (fully unit-stride writes):
            # per box: (M(x0,x2), M(y0,y2), M(x1,x3), M(y1,y3)).
            if l1_dense:
                # dense shift-by-2: every operand fully unit-stride; useful
                # lanes at 8j (x-pair max), 8j+1 (y-pair), 8j+4, 8j+5.
                w = 8 * b - 2
                cdf = st["cd"][:]
                mx = l1pool.tile([p, 8 * b], mid, tag="mx")
                mn = l1pool.tile([p, 8 * b], mid, tag="mn")
                nc.vector.tensor_tensor(out=mx[:, 0:w], in0=cdf[:, 0:w],
                                        in1=cdf[:, 2 : 8 * b], op=Alu.max)
                nc.vector.tensor_tensor(out=mn[:, 0:w], in0=cdf[:, 0:w],
                                        in1=cdf[:, 2 : 8 * b], op=Alu.min)
            else:
                mx = l1pool.tile([p, 4 * b], mid, tag="mx")
                mn = l1pool.tile([p, 4 * b], mid, tag="mn")
                lo = cdh[:, :, 0, :]
                hi = cdh[:, :, 1, :]
                nc.vector.tensor_tensor(
                    out=mx[:].rearrange("p (b k) -> p b k", k=4), in0=lo, in1=hi,
                    op=Alu.max,
                )
                nc.vector.tensor_tensor(
                    out=mn[:].rearrange("p (b k) -> p b k", k=4), in0=lo, in1=hi,
                    op=Alu.min,
                )
            st["mx"], st["mn"] = mx, mn

        def s2_l2(t):
            f, s = divmod(t, sub)
            st = state[t]
            if l1_dense:
                mxv = st["mx"][:].rearrange("p (b k) -> p b k", k=8)[:, :, 0:6]
                mnv = st["mn"][:].rearrange("p (b k) -> p b k", k=8)[:, :, 0:6]
                sel0, sel1 = (0, 2), (4, 6)
            else:
                mxv = st["mx"][:].rearrange("p (b k) -> p b k", k=4)
                mnv = st["mn"][:].rearrange("p (b k) -> p b k", k=4)
                sel0, sel1 = (0, 2), (2, 4)
            # L2 -> interleaved (x, y) pair vectors [p, 2b], contiguous.
            xy1 = spool.tile([p, 2 * b], mid, tag="xy1")  # (xb1, yb1) pairs
            xy2 = spool.tile([p, 2 * b], mid, tag="xy2")  # (xb2, yb2) pairs
            nc.vector.tensor_tensor(
                out=xy1[:].rearrange("p (b two) -> p b two", two=2),
                in0=mxv[:, :, sel0[0]:sel0[1]], in1=mxv[:, :, sel1[0]:sel1[1]], op=Alu.max,
            )
            nc.vector.tensor_tensor(
                out=xy2[:].rearrange("p (b two) -> p b two", two=2),
                in0=mnv[:, :, sel0[0]:sel0[1]], in1=mnv[:, :, sel1[0]:sel1[1]], op=Alu.min,
            )
            # mask -> {0 valid, -1e30 masked}, duplicated per (x,y) lane
            maskm = spool.tile([p, 2 * b], mid, tag="maskm")
            msrc = state[("m", f)][:, s * b : (s + 1) * b]
            nc.scalar.activation(
                out=maskm[:].rearrange("p (b two) -> p b two", two=2),
                in_=msrc.rearrange("p (b one) -> p b one", one=1).broadcast_to((p, b, 2)),
                func=Act.Copy, bias=-1e30, scale=1e30,
            )
            st.update(xy1=xy1, xy2=xy2, maskm=maskm)

        def s3(t):
            f, s = divmod(t, sub)
            st = state[t]
            ehi = egob_sb[:, 4 * f : 4 * f + 2].rearrange(
                "p (one two) -> p one two", one=1).broadcast_to((p, b, 2))
            # pm = min((xb1,yb1), (xa1,ya1)) ; pmm = pm + maskm
            pm = spool.tile([p, 2 * b], mid, tag="pm")
            nc.vector.tensor_tensor(
                out=pm[:].rearrange("p (b two) -> p b two", two=2),
                in0=st["xy1"][:].rearrange("p (b two) -> p b two", two=2),
                in1=ehi, op=Alu.min,
            )
            pmm = spool.tile([p, 2 * b], mid, tag="pmm")
            nc.vector.tensor_tensor(out=pmm[:], in0=pm[:], in1=st["maskm"][:],
                                    op=Alu.add)
            st["pmm"] = pmm

        def s4(t):
            f, s = divmod(t, sub)
            st = state[t]
            elo = egob_sb[:, 4 * f + 2 : 4 * f + 4].rearrange(
                "p (one two) -> p one two", one=1).broadcast_to((p, b, 2))
            qm = spool.tile([p, 2 * b], mid, tag="qm")
            nc.vector.tensor_tensor(
                out=qm[:].rearrange("p (b two) -> p b two", two=2),
                in0=st["xy2"][:].rearrange("p (b two) -> p b two", two=2),
                in1=elo, op=Alu.max,
            )
            negm = spool.tile([p, 2 * b], mid, tag="negm")
            nc.vector.tensor_tensor(out=negm[:], in0=qm[:], in1=st["pmm"][:],
                                    op=Alu.subtract)
            st["negm"] = negm

        def s5(t):
            st = state[t]
            # pos = relu(-negm) = (wpos_masked, hpos) interleaved
            pos = spool.tile([p, 2 * b], mid, tag="pos")
            nc.scalar.activation(out=pos[:], in_=st["negm"][:], func=Act.Relu,
                                 scale=-1.0)
            st["pos"] = pos

        def s6(t):
            st = state[t]
            # area = wpos * hpos (even * odd lanes), accumulated per
            # partition into acc column.
            posv = st["pos"][:].rearrange("p (b two) -> p b two", two=2)
            scr = spool.tile([p, b], mid, tag="scr")
            nc.vector.scalar_tensor_tensor(
                out=scr[:], in0=posv[:, :, 0], scalar=0.0, in1=posv[:, :, 1],
                op0=Alu.bypass, op1=Alu.mult,
                accum_out=acc[:, t : t + 1],
            )
            del state[t]

        # 7-stage software pipeline: every cross-engine hop of the tail
        # chain lands in its own period, so no in-order engine queue ever
        # blocks on a same-subtile dependency.
        stages = [s0_dma, s1_l1, s2_l2, s3, s4, s5, s6]
        for t in range(n_tiles + len(stages) - 1):
            for k, fn in enumerate(stages):
                tt = t - k
                if 0 <= tt < n_tiles:
                    fn(tt)

        total = const_pool.tile([p, 1], f32)
        nc.vector.reduce_sum(out=total[:], in_=acc[:], axis=mybir.AxisListType.X)
        nc.sync.dma_start(out=out.ap(), in_=total[:])

    nc.compile()
    return nc


def _get_prog():
    global _prog
    if _prog is None:
        _prog = _build_program()
    return _prog


def _ego_aabb(sdc_traj_all, sdc_planning_gt):
    """Per-future ego AABB [F,4] = (xa1, xa2, ya1, ya2), mirroring reference."""
    sdc_traj_all = np.asarray(sdc_traj_all, dtype=np.float32)
    sdc_planning_gt = np.asarray(sdc_planning_gt, dtype=np.float32)
    x = sdc_traj_all[0, :, 0]
    y = sdc_traj_all[0, :, 1]
    theta = sdc_planning_gt[0, :, 2]
    local = np.array(
        [[W / 2, -H / 2], [W / 2, H / 2], [-W / 2, H / 2], [-W / 2, -H / 2]],
        dtype=np.float32,
    )
    c, s = np.cos(theta), np.sin(theta)
    rot = np.stack([np.stack([c, s], -1), np.stack([-s, c], -1)], -2)  # [F,2,2]
    corners = np.einsum("fij,kj->fki", rot, local) + np.stack([x, y], -1)[:, None, :]
    corners = corners.astype(np.float32)
    xa1 = corners[..., 0].max(-1)
    ya1 = corners[..., 1].max(-1)
    xa2 = corners[..., 0].min(-1)
    ya2 = corners[..., 1].min(-1)
    return np.stack([xa1, xa2, ya1, ya2], -1).astype(np.float32)  # [F,4]


def kernel(sdc_traj_all, sdc_planning_gt, sdc_planning_gt_mask, future_gt_corners, box_mask):
    from concourse.bass_utils import run_bass_kernel_spmd

    corners = np.asarray(future_gt_corners, dtype=np.float32).reshape(F, N, 8)
    mask = np.asarray(box_mask)
    if mask.dtype == np.bool_:
        mask_u8 = mask.view(np.uint8)
    else:
        mask_u8 = (mask != 0).astype(np.uint8)

    eg = _ego_aabb(sdc_traj_all, sdc_planning_gt)  # [F,4] = (xa1, xa2, ya1, ya2)
    ego_arr = np.ascontiguousarray(
        np.broadcast_to(eg.reshape(4 * F), (P, 4 * F)), dtype=np.float32
    )
    import ml_dtypes
    # pair layout per future: (xa1, ya1, xa2, ya2)
    egp = np.stack([eg[:, 0], eg[:, 2], eg[:, 1], eg[:, 3]], -1).reshape(4 * F)
    egob_arr = np.ascontiguousarray(
        np.broadcast_to(egp, (P, 4 * F))
    ).astype(ml_dtypes.bfloat16)

    in_maps = []
    for cidx in range(CORES):
        lo, hi = cidx * PER_CORE, (cidx + 1) * PER_CORE
        m = {"ego": ego_arr, "egob": egob_arr}
        for f in range(F):
            m[f"corners{f}"] = corners[f, lo:hi]
            m[f"mask{f}"] = mask_u8[f, lo:hi]
        in_maps.append(m)

    global _last_in_maps
    _last_in_maps = in_maps
    res = run_bass_kernel_spmd(_get_prog(), in_maps, list(range(CORES))).results
    total = 0.0
    for r in res:
        total += float(r["out"].astype(np.float64).sum())
    return np.array([total], dtype=np.float32) * np.float32(WEIGHT)
